# revision 39
# baseline (speedup 1.0000x reference)
"""Batched DWI 3D conv as implicit GEMM on 8 TRN2 NeuronCores.

Problem: x [8, 12, 12, 32, 32, 32] f32, W [32, 12, 12, 3, 3, 3] f32
         -> out [8, 32, 30, 30, 30] f32  (VALID 3D conv, c_in = 144)

Strategy (data-parallel over batch, one batch element per core):
  - x viewed as [144, 32, 32, 32] bf16 in SBUF; a kernel offset (dx, dy, dz)
    is a pure free-dim shift, so the conv is a chain of shifted matmuls
    accumulated in PSUM: out[co, n] += W_d^T @ x[:, n + shift(d)].
  - c_out = 32 fills only 1/4 of the PE array columns, so 4 col-tiled
    matmuls run concurrently (tile_position=(0, 32j)), each computing a
    different output chunk into its own 32-partition PSUM slice. Measured:
    a 4-matmul phase streams in ~190 ns = one N=450 pass, LDWEIGHTS hidden.
  - c_in = 144 = 128 + 16. The 16-channel tail is packed as a host-built
    [96, .] tile of shifted copies: blocks (dyb in {0,1}) x (dz in {0,1,2})
    hold the tail shifted by 32*dyb + dz. Two tail matmuls per dx cover all
    nine (dy, dz) offsets: phase 0 reads at the chunk base (covers dy 0,1),
    phase 1 reads one y-row down so the dyb=1 blocks cover dy=2 while the
    dyb=0 blocks (which would duplicate dy=1) get zero weights.
  - Per output chunk (fixed ox, 15 y-rows, 30 z): 27 full matmuls (K=128)
    + 6 tail matmuls (K=96) accumulate into one PSUM-bank slice, N = 450.
  - DMAs sized for >=16KB per-partition records (descriptor overhead on HBM
    is ~0.4 us) and >=65 partitions (a 16-partition transfer only engages
    ~4 of 16 SDMA engines); loads are interleaved so each piece lands just
    before the group that consumes it; output staged per group and stored
    with contiguous 7.2KB records; dummy matmuls warm the PE clock (HAM)
    during the initial load window.
  Measured: ~131 us on 8 cores (PE phase floor ~94 us + load ramp + drain).
  All DMAs stay on the sync HWDGE ring: moving any traffic to the ACT
  HWDGE ring or gpsimd SWDGE measurably degrades aggregate bandwidth.
"""

import numpy as np
import ml_dtypes

import concourse.bass as bass
import concourse.bacc as bacc
import concourse.mybir as mybir
import concourse.tile as tile
from concourse.bass_utils import run_bass_kernel_spmd

BF16 = mybir.dt.bfloat16
F32 = mybir.dt.float32

N_CORES = 8
CIN = 144
COUT = 32
DIM = 32
ODIM = 30
NCH = 450  # one chunk = 15 y-rows x 30 z
NFLAT = DIM * DIM * DIM
CHUNKS = [(ox, h) for ox in range(ODIM) for h in (0, 1)]  # 60 chunks
# chunks per group (each group = nbank psum banks x 4 col tiles); group sizes
# ramp up so the PE starts after only the first small x slab lands
GROUP_SIZES = [4, 8, 16, 16, 12, 4]
# x body loaded in plane slabs sized to stay ahead of PE consumption
XSLABS = [(0, 4), (4, 8), (8, 16), (16, 24), (24, 32)]
T_ROWS = 96  # 6 blocks of 16: (dyb in {0,1}) x (dz in {0,1,2}), shift 32*dyb+dz

_CACHE = {}


def _ctiles():
    out = []
    for dx in range(3):
        for dy in range(3):
            for dz in range(3):
                out.append(("full", dx, dy, dz))
    for dx in range(3):
        for ph in range(2):
            out.append(("tail", dx, ph, 0))
    return out


def build_nc():
    nc = bacc.Bacc(None, target_bir_lowering=False)
    xin = nc.dram_tensor("x", [128, DIM, DIM, DIM], BF16, kind="ExternalInput")
    # host-prepared dz-shifted tail (65 partitions so the DMA spreads across
    # many SDMA engines; a 16-partition transfer would use only ~4 of 16)
    xt_d = nc.dram_tensor("xt", [T_ROWS, DIM, DIM, DIM], BF16, kind="ExternalInput")
    wf_d = nc.dram_tensor("wf", [128, 27, COUT], BF16, kind="ExternalInput")
    wt_d = nc.dram_tensor("wt", [T_ROWS, 6, COUT], BF16, kind="ExternalInput")
    # output laid out [partition = 32*colgroup + co, bank_seq, 450] so each
    # group's store is one DMA with contiguous per-partition records
    n_banks_total = sum(g // 4 for g in GROUP_SIZES)
    out_d = nc.dram_tensor("out", [128, n_banks_total, NCH], F32, kind="ExternalOutput")

    ctiles = _ctiles()
    last = len(ctiles) - 1

    with tile.TileContext(nc) as tc:
        with (
            tc.tile_pool(name="wpool", bufs=1) as wpool,
            tc.tile_pool(name="xpool", bufs=1) as xpool,
            tc.tile_pool(name="tpool", bufs=1) as tpool,
            tc.tile_pool(name="spool", bufs=3) as spool,
            tc.tile_pool(name="ppool", bufs=8, space="PSUM") as ppool,
        ):
            WF = wpool.tile([128, 27, COUT], BF16, tag="wf")
            WT = wpool.tile([T_ROWS, 6, COUT], BF16, tag="wt")
            nc.sync.dma_start(WF[:], wf_d[:])
            nc.sync.dma_start(WT[:], wt_d[:])

            XPG = []
            for si, (p0, p1) in enumerate(XSLABS):
                t = xpool.tile([128, p1 - p0, DIM, DIM], BF16, tag=f"xp{si}")
                XPG.append(t)
            T = tpool.tile([T_ROWS, DIM, DIM, DIM], BF16, tag="tail")

            def load_tail_q(qi):
                a, b = qi * (DIM // 4), (qi + 1) * (DIM // 4)
                nc.sync.dma_start(T[:, a:b, :, :], xt_d[:, a:b, :, :])

            def load_slab(si, eng):
                p0, p1 = XSLABS[si]
                eng.dma_start(XPG[si][:], xin[:, p0:p1, :, :])

            # issue order = completion order: each piece lands just ahead
            # of the group that first consumes it
            load_slab(0, nc.sync)      # planes 0-3
            load_slab(1, nc.sync)      # planes 4-7
            load_tail_q(0)             # tail planes 0-7
            load_slab(2, nc.sync)      # planes 8-15
            load_tail_q(1)             # tail planes 8-15
            load_slab(3, nc.sync)      # planes 16-23
            load_slab(4, nc.sync)      # planes 24-31
            # back half of the tail has loose deadlines: one merged DMA
            nc.sync.dma_start(T[:, 16:32, :, :], xt_d[:, 16:32, :, :])

            # warm up the PE (HAM clock gate) during the initial load
            # window: dummy matmuls on never-written tiles have no deps,
            # so they run immediately; their PSUM bank is reused later
            # with start=True which clears it
            warm = wpool.tile([128, 512], BF16, tag="warm")
            nc.gpsimd.memset(warm[:], 0.0)
            pwarm = ppool.tile([128, 450], F32, tag="ps", name="ps_warm")
            for wi in range(24):
                nc.tensor.matmul(pwarm[0:32, :], warm[:, 0:32], warm[:, 32:482],
                                 start=(wi == 0), stop=(wi == 23),
                                 tile_position=(0, 0))

            def xplane(p):
                for si, (p0, p1) in enumerate(XSLABS):
                    if p < p1:
                        return XPG[si], p - p0
                raise AssertionError

            g0 = 0
            nb0 = 0  # running bank counter (output bank_seq index)
            for gi, gsz in enumerate(GROUP_SIZES):
                gch = CHUNKS[g0 : g0 + gsz]
                nbank = len(gch) // 4
                ptiles = [ppool.tile([128, NCH], F32, tag="ps", name=f"ps_{gi}_{bi}")
                          for bi in range(nbank)]
                for t, (kind, dx, dy, dz) in enumerate(ctiles):
                    if kind == "full":
                        lhsT = WF[:, dx * 9 + dy * 3 + dz, :]
                    else:
                        lhsT = WT[:, dx * 2 + dy, :]  # dy is the phase here
                    for bi in range(nbank):
                        P = ptiles[bi]
                        for j in range(4):
                            ox, h = gch[bi * 4 + j]
                            y0 = 15 * h
                            p = ox + dx
                            if kind == "full":
                                xt, lp = xplane(p)
                                rhs = xt[:, lp, y0 + dy : y0 + dy + 15, dz : dz + 30]
                            else:
                                # tail phase ph=dy: AP shifted by ph y-rows;
                                # phase 1 zero-weights the dy=1 duplicate rows
                                rhs = T[:, p, y0 + dy : y0 + dy + 15, 0:30]
                            nc.tensor.matmul(
                                P[32 * j : 32 * (j + 1), :],
                                lhsT,
                                rhs,
                                start=(t == 0),
                                stop=(t == last),
                                tile_position=(0, 32 * j),
                            )
                st = spool.tile([128, nbank * NCH], F32, tag="st",
                                padded_shape=[128, 4 * NCH], name=f"st_{gi}")
                for bi in range(nbank):
                    nc.vector.tensor_copy(st[:, bi * NCH : (bi + 1) * NCH],
                                          ptiles[bi][:])
                # stores ride the ACT HWDGE ring: they have no deadline
                # (spool bufs give ~3 groups of slack), so this keeps store
                # packets out of the sync ring competing with critical loads
                nc.scalar.dma_start(out_d[:, nb0 : nb0 + nbank, :], st[:])
                g0 += gsz
                nb0 += nbank

    nc.compile()
    return nc


def _get_nc():
    if "nc" not in _CACHE:
        _CACHE["nc"] = build_nc()
    return _CACHE["nc"]


def _prep_inputs(x, W):
    bf16 = ml_dtypes.bfloat16
    xr = np.ascontiguousarray(x.reshape(8, CIN, DIM, DIM, DIM)).astype(bf16)
    Wr = W.reshape(COUT, CIN, 3, 3, 3).astype(np.float32)

    # host-built shifted tail: block (dyb, dz) holds tail shifted by 32*dyb+dz
    tails = xr[:, 128:144].reshape(8, 16, NFLAT)
    xt = np.zeros((8, T_ROWS, NFLAT), bf16)
    for dyb in range(2):
        for dz in range(3):
            s = 32 * dyb + dz
            blk = (dyb * 3 + dz) * 16
            xt[:, blk : blk + 16, 0 : NFLAT - s] = tails[:, :, s:]
    xt = xt.reshape(8, T_ROWS, DIM, DIM, DIM)

    wf = np.ascontiguousarray(
        Wr[:, :128].reshape(COUT, 128, 27).transpose(1, 2, 0)
    ).astype(bf16)

    # tail weights per (dx, phase): phase 0 covers dy = dyb (0,1);
    # phase 1 re-reads with a +1 y-row AP shift, so dyb=1 rows cover dy=2
    # and dyb=0 rows (which would duplicate dy=1) get zero weights
    wt = np.zeros((T_ROWS, 6, COUT), np.float32)
    tailW = Wr[:, 128:144]  # [co, t, dx, dy, dz]
    for dx in range(3):
        for dyb in range(2):
            for dz in range(3):
                blk = (dyb * 3 + dz) * 16
                wt[blk : blk + 16, dx * 2 + 0] = tailW[:, :, dx, dyb, dz].T
                if dyb == 1:
                    wt[blk : blk + 16, dx * 2 + 1] = tailW[:, :, dx, 2, dz].T
    wt = wt.astype(bf16)

    return [{"x": np.ascontiguousarray(xr[b, :128]), "xt": xt[b], "wf": wf, "wt": wt}
            for b in range(N_CORES)]


def kernel(x, W, _trace=False):
    nc = _get_nc()
    in_maps = _prep_inputs(np.asarray(x), np.asarray(W))
    res = None
    for attempt in range(3):
        try:
            res = run_bass_kernel_spmd(nc, in_maps, list(range(N_CORES)), trace=_trace)
            break
        except Exception:
            # rare transient NRT_EXEC_UNIT_UNRECOVERABLE flakes; retry
            if attempt == 2:
                raise
            import time as _time
            _time.sleep(2.0)
    full = np.empty((N_CORES, COUT, ODIM, ODIM, ODIM), np.float32)
    for b in range(N_CORES):
        o = res.results[b]["out"]  # [128, n_banks_total, 450]
        nb = 0
        g0 = 0
        for gsz in GROUP_SIZES:
            for bi in range(gsz // 4):
                for j in range(4):
                    ox, h = CHUNKS[g0 + 4 * bi + j]
                    full[b, :, ox, 15 * h : 15 * h + 15, :] = (
                        o[32 * j : 32 * j + 32, nb].reshape(COUT, 15, 30))
                nb += 1
            g0 += gsz
    if _trace:
        return full, res
    return full
